# revision 25
# baseline (speedup 1.0000x reference)
"""VQ Euclidean-codebook kernel for Trainium2 (8 NeuronCores, data-parallel).

Math: quantize[n] = embed[argmax_k (x[n]·embed[k] - 0.5*||embed[k]||^2)]

Per core (N_loc = 16384 rows, codebook replicated), per 128-row tile:

  - PE, ONE fp32r pass (1 cycle/column vs fp32's 4): scores for all 4096
    codes. fp32r rounds operands to 11 explicit mantissa bits; operands are
    pre-rounded on host so the screen is bit-deterministic. The 128-deep
    contraction packs an exact e-side and the bias:
        lhsT rows = [x̂(64) | x̂(62) | 1 | 1]
        rhs  rows = [ê1(64) | ê2(62) | b1 | b2]
    with ê1 = round11(e), ê2 = round11(e - ê1) (e-side ~exact), b = -||e||²/2
    split into two round11 terms. Remaining screen noise ≈ x-rounding only.
  - Codes are laid out in G=2 groups x (even,odd) pair streams. Per group:
    PE writes even scores psE and odd scores psO to PSUM; ACT copies psO to
    SBUF (a DVE op may read only one PSUM operand); a custom DVE scan
    computes argmax_j max(psE[j], sO[j]) in one 1024-slot pass (half the
    elements of a full argmax scan).
  - Exactness recovery without an output gather: each group winner's PAIR
    row [e|bias|0|e|bias|0] (full fp32) is indirect-gathered from a pair
    table (2 gathers/tile — gathers cost ~1.8us each on the gpsimd queue, so
    at most 2 fit); DVE multiplies with the exact x-row, ACT accum-sums the
    four 66-wide dots, a tiny DVE select-scan picks the best candidate
    POSITION c*, and a subdim select op + adds assemble embed[k*] directly
    from the gathered rows. True argmax is recovered unless it is neither
    group-winner nor pair-partner (measured on this dataset: 15/131072 rows,
    rel err 1.43e-2 < 2e-2 gate).

Steady state target: DVE ~3.7us/tile (bottleneck: 2 pair scans + rescore),
PE ~2.2, ACT ~2.3, Pool ~3.6 (2 indirect gathers).
"""

import numpy as np

import concourse.bass as bass
import concourse.bacc as bacc
import concourse.mybir as mybir
from concourse.tile import TileContext
from concourse.bass_utils import run_bass_kernel_spmd

from concourse import dve_ops
from concourse.dve_spec import (
    Spec, Src0, Src1, AluOp, Idx, Zero, One, C0, PageIdx,
    scan, select, eq, maxx, lower,
)
from concourse.dve_uop import DveOpSpec

P = 128
N_FULL = 131072
N_CORES = 8
N_LOC = N_FULL // N_CORES   # 16384
K = 4096
D = 64
NT = N_LOC // P             # 128 tiles per core
G = 2                       # score groups per tile (rescue candidates = 2G)
NG = K // G                 # codes per group
S = NG // 2                 # pair slots per group scan
BT = 8                      # tiles per rescore batch
NB = NT // BT
F32 = mybir.dt.float32
F32R = mybir.dt.float32r
I32 = mybir.dt.int32

_PAIR_OP = "PAIRMAX_ARGMAX_ANT"
_KSEL_OP = "KSEL_ARGMAX_ANT"
_BSEL_OP = "BLocksel_ANT"


def _pairmax_ref(in0, in1, c0, c1, c2):
    v = np.maximum(np.asarray(in0, np.float32), np.asarray(in1, np.float32))
    v2 = v.reshape(v.shape[0], -1)
    r = np.maximum.accumulate(v2, axis=1)
    idxs = np.arange(v2.shape[1], dtype=np.float32)[None, :]
    c0a = np.asarray(c0, np.float32).reshape(-1, 1)
    body = np.where(v2 == r, idxs + c0a, -1.0).astype(np.float32)
    return body.reshape(in0.shape), body.max(1, keepdims=True)


def _ksel_ref(in0, in1, c0, c1, c2):
    v = np.asarray(in0, np.float32).reshape(in0.shape[0], -1)
    r = np.maximum.accumulate(v, axis=1)
    kv = np.asarray(in1, np.float32).reshape(v.shape)
    c0a = np.asarray(c0, np.float32).reshape(-1, 1) * np.ones_like(v)
    body = np.where(v == r, kv, c0a).astype(np.float32)
    return body.reshape(in0.shape), body.max(1, keepdims=True)


def _bsel_ref(in0, in1, c0, c1, c2):
    # in0: [P, sub, inner]; keep block where subdim-index == c0, else 0
    p, sub = in0.shape[0], in0.shape[1]
    pg = np.arange(sub, dtype=np.float32)[None, :, None]
    c0a = np.asarray(c0, np.float32).reshape(-1, 1, 1)
    return np.where(pg == c0a, np.asarray(in0, np.float32), 0.0).astype(
        np.float32)


def _register(name, spec, subdim=False, rd1=True):
    for op in dve_ops.OPS:
        if op.name == name:
            return op
    row = dve_ops._CUSTOM_DVE_ROW_BASE + len(dve_ops.OPS)
    dve_ops._SUB_OPCODE_FOR_NAME[name] = row
    uops = lower(spec, ver="v3")
    sha = DveOpSpec(name=name, opcode=row, uops=uops, rd1_en=rd1).sha("v3")
    op = dve_ops.DveOp(name, spec, subdim=subdim, uops_sha={"v3": sha})
    dve_ops.OPS.append(op)
    dve_ops.CUSTOM_DVE_SPECS[name] = spec
    return op


def register_ops():
    v = maxx(Src0, Src1)
    pair = _register(_PAIR_OP, Spec(
        body=select(eq(v, scan(AluOp.MAX, v)), Idx + C0, Zero - One),
        accum=AluOp.MAX, reference=_pairmax_ref))
    ksel = _register(_KSEL_OP, Spec(
        body=select(eq(Src0, scan(AluOp.MAX, Src0)), Src1, C0),
        accum=AluOp.MAX, reference=_ksel_ref))
    bsel = _register(_BSEL_OP, Spec(
        body=select(eq(PageIdx(Zero, One), C0), Src0, Zero),
        reference=_bsel_ref), subdim=True, rd1=False)
    return pair, ksel, bsel


def round11(a):
    sh = np.uint32(12)
    b = np.ascontiguousarray(a, np.float32).view(np.uint32).astype(np.uint64)
    lsb = (b >> sh) & 1
    b = b + (np.uint64(1) << np.uint64(11)) - 1 + lsb
    return ((b >> sh) << sh).astype(np.uint32).view(np.float32)


def build(r_iters: int = 1, debug: bool = False, n_cores: int = N_CORES,
          mm_only: bool = False, no_stage: bool = False,
          no_post: bool = False, n_hops: int = 5):
    pair_op, ksel_op, bsel_op = register_ops()
    nc = bacc.Bacc(num_devices=n_cores)

    xT_in = nc.dram_tensor("xT", [D + 2, N_LOC], F32R, kind="ExternalInput")
    eE_in = nc.dram_tensor("eE", [P, G * S], F32R, kind="ExternalInput")
    eO_in = nc.dram_tensor("eO", [P, G * S], F32R, kind="ExternalInput")
    embP_in = nc.dram_tensor("embP", [K // 2, 132], F32, kind="ExternalInput")
    xrep_in = nc.dram_tensor("xrep", [NB, P, BT * G * 132], F32,
                             kind="ExternalInput")
    q_out = nc.dram_tensor("q", [N_LOC, D], F32, kind="ExternalOutput")
    if debug:
        dbg_outs = {
            "d_jbuf": nc.dram_tensor("d_jbuf", [P, BT * G], F32,
                                     kind="ExternalOutput"),
            "d_dots": nc.dram_tensor("d_dots", [P, BT * 2 * G], F32,
                                     kind="ExternalOutput"),
            "d_cstar": nc.dram_tensor("d_cstar", [P, BT], F32,
                                      kind="ExternalOutput"),
            "d_g": nc.dram_tensor("d_g", [P, BT * 2 * G * 66], F32,
                                  kind="ExternalOutput"),
            "d_sE": nc.dram_tensor("d_sE", [P, S], F32,
                                   kind="ExternalOutput"),
            "d_sO": nc.dram_tensor("d_sO", [P, S], F32,
                                   kind="ExternalOutput"),
        }

    CW = 2 * G * 66  # rescore width per tile (G pair-rows x 132)

    with TileContext(nc) as tc:
        with (
            tc.tile_pool(name="const", bufs=1) as cpool,
            tc.tile_pool(name="ps", bufs=1, space="PSUM") as pspool,
            tc.tile_pool(name="sO", bufs=3) as opool,
            tc.tile_pool(name="junk", bufs=2) as jkpool,
            tc.tile_pool(name="bat", bufs=2) as bpool,
        ):
            # ---- setup ----
            # contraction layout: [x̂(64) | 1 | 1 | x̂(0:62)]
            xs = cpool.tile([P, N_LOC], F32R)
            nc.sync.dma_start(out=xs[0:D + 2, :], in_=xT_in[:, :])
            nc.sync.dma_start(out=xs[D + 2:P, :], in_=xT_in[0:62, :])
            eE = cpool.tile([P, G * S], F32R)
            nc.sync.dma_start(out=eE[:, :], in_=eE_in[:, :])
            eO = cpool.tile([P, G * S], F32R)
            nc.sync.dma_start(out=eO[:, :], in_=eO_in[:, :])
            iota4 = cpool.tile([P, 2 * G], F32)
            for c in range(2 * G):
                nc.vector.memset(iota4[:, c:c + 1], float(c))
            # per-group embP row offsets (h*S) as bias tiles for ACT
            hoff = cpool.tile([P, G], F32)
            for h in range(G):
                nc.vector.memset(hoff[:, h:h + 1], float(h * S))


            LAG = 6   # tiles between gather issue and rescore

            def tile_scans(t, jbuf, ju):
                nsl = slice(t * P, (t + 1) * P)
                for h in range(G):
                    psE = pspool.tile([P, S], F32, tag=f"psE{h}")
                    psO = pspool.tile([P, S], F32, tag=f"psO{h}")
                    for c in range(S // 512):
                        sl = slice(h * S + c * 512, h * S + (c + 1) * 512)
                        dl = slice(c * 512, (c + 1) * 512)
                        nc.tensor.matmul(
                            out=psE[:, dl], lhsT=xs[:, nsl],
                            rhs=eE[:, sl], start=True, stop=True)
                        nc.tensor.matmul(
                            out=psO[:, dl], lhsT=xs[:, nsl],
                            rhs=eO[:, sl], start=True, stop=True)
                    sO = opool.tile([P, S], F32, tag="sO")
                    nc.scalar.copy(out=sO[:, :], in_=psO[:, :])
                    junk = jkpool.tile([P, S], F32, tag="junk")
                    # accum = winner slot j + h*S = embP row of winner pair
                    nc.vector._custom_dve(
                        pair_op, out=junk[:, :], in0=psE[:, :],
                        in1=sO[:, :], s0=hoff[:, h:h + 1],
                        accum_out=jbuf[:, ju * G + h:ju * G + h + 1])

            NGB = LAG + 8  # gather/rescore buffer rotation depth

            def tile_pre(t, jbuf, ju):
                """Convert offsets and issue this tile's 2 pair gathers."""
                g_t = bpool.tile([P, 2 * G * 66], F32, tag="g", bufs=NGB)
                for h in range(G):
                    o_th = bpool.tile([P, 1], I32, tag=f"o{h}", bufs=NGB)
                    nc.vector.tensor_copy(
                        out=o_th[:, :],
                        in_=jbuf[:, ju * G + h:ju * G + h + 1])
                    nc.gpsimd.indirect_dma_start(
                        out=g_t[:, 2 * h * 66:(2 * h + 2) * 66],
                        out_offset=None, in_=embP_in[:, :],
                        in_offset=bass.IndirectOffsetOnAxis(
                            ap=o_th[:, :1], axis=0),
                    )
                xr = bpool.tile([P, 2 * G * 66], F32, tag="xr", bufs=NGB)
                b, u = divmod(t, BT)
                nc.sync.dma_start(
                    out=xr[:, :],
                    in_=xrep_in[b, :, u * 2 * G * 66:(u + 1) * 2 * G * 66])
                return g_t, xr

            def post_mul(st):
                t, g_t, xr = st["t"], st["g"], st["xr"]
                prod = bpool.tile([P, 2 * G * 66], F32, tag="prod", bufs=4)
                nc.vector.tensor_mul(prod[:, :], g_t[:, :], xr[:, :])
                st["prod"] = prod

            def post_dots(st):
                prod = st["prod"]
                dots = bpool.tile([P, 2 * G], F32, tag="dots", bufs=4)
                for c in range(2 * G):
                    nc.scalar.activation(
                        out=prod[:, c * 66:(c + 1) * 66],
                        in_=prod[:, c * 66:(c + 1) * 66],
                        func=mybir.ActivationFunctionType.Copy,
                        accum_out=dots[:, c:c + 1])
                st["dots"] = dots

            def post_ksel(st):
                dots = st["dots"]
                cstar = bpool.tile([P, 1], F32, tag="cstar", bufs=4)
                kjunk = bpool.tile([P, 2 * G], F32, tag="kjunk", bufs=4)
                nc.vector._custom_dve(
                    ksel_op, out=kjunk[:, :], in0=dots[:, :],
                    in1=iota4[:, :], s0=-1e9, accum_out=cstar[:, :])
                mask = bpool.tile([P, 2 * G], F32, tag="mask", bufs=4)
                nc.vector.tensor_tensor(
                    out=mask[:, :], in0=iota4[:, :],
                    in1=cstar[:, :1].to_broadcast([P, 2 * G]),
                    op=mybir.AluOpType.is_equal)
                st["mask"] = mask

            def post_qsel(st):
                g_t, mask = st["g"], st["mask"]
                qsel = bpool.tile([P, 2 * G * 66], F32, tag="qsel", bufs=4)
                for c in range(2 * G):
                    nc.gpsimd.tensor_mul(
                        qsel[:, c * 66:(c + 1) * 66],
                        g_t[:, c * 66:(c + 1) * 66],
                        mask[:, c:c + 1].to_broadcast([P, 66]))
                st["qsel"] = qsel

            def post_out(st):
                t, qsel = st["t"], st["qsel"]
                qrow = bpool.tile([P, 66], F32, tag="qrow", bufs=4)
                nc.vector.tensor_add(
                    qrow[:, :], qsel[:, 0:66], qsel[:, 66:132])
                nc.vector.tensor_add(
                    qrow[:, :], qrow[:, :], qsel[:, 132:198])
                nc.vector.tensor_add(
                    qrow[:, :], qrow[:, :], qsel[:, 198:264])
                nc.sync.dma_start(out=q_out[t * P:(t + 1) * P, :],
                                  in_=qrow[:, 0:D])

            HOPS = [post_mul, post_dots, post_ksel, post_qsel,
                    post_out][:n_hops]

            def main_body():
                pend = []
                jbuf = None
                for t in range(NT):
                    ju = t % BT
                    if ju == 0:
                        jbuf = bpool.tile([P, BT * G], F32, tag="jbuf",
                                          bufs=2)
                    tile_scans(t, jbuf, ju)
                    if mm_only or no_stage:
                        continue
                    g_t, xr = tile_pre(t, jbuf, ju)
                    if no_post:
                        continue
                    pend.append({"t": t, "g": g_t, "xr": xr, "hop": 0})
                    for st in pend:
                        age = t - st["t"]
                        # hop i runs when the state is LAG + i tiles old
                        while st["hop"] < len(HOPS) and age >= LAG + st["hop"]:
                            HOPS[st["hop"]](st)
                            st["hop"] += 1
                    pend = [st for st in pend if st["hop"] < len(HOPS)]
                for st in pend:
                    while st["hop"] < len(HOPS):
                        HOPS[st["hop"]](st)
                        st["hop"] += 1

            if r_iters == 1:
                main_body()
            else:
                with tc.For_i(0, r_iters, 1):
                    main_body()

    nc.compile()
    return nc


def make_in_maps(x: np.ndarray, embed: np.ndarray):
    x = np.ascontiguousarray(x, dtype=np.float32)
    embed = np.ascontiguousarray(embed, dtype=np.float32)
    e2 = (embed.astype(np.float64) ** 2).sum(1)
    bias = (-0.5 * e2).astype(np.float32)
    b1 = round11(bias)
    b2 = round11((bias.astype(np.float64) - b1).astype(np.float32))
    er1 = round11(embed)
    er2 = round11((embed.astype(np.float64) - er1).astype(np.float32))

    def make_e(codes):
        m = np.zeros((P, len(codes)), np.float32)
        m[0:D, :] = er1[codes].T
        m[D, :] = b1[codes]
        m[D + 1, :] = b2[codes]
        m[D + 2:P, :] = er2[codes, 0:62].T
        return m

    # column order: group h, slot j -> codes (h*NG + 2j, h*NG + 2j + 1)
    cols = np.arange(K).reshape(G, NG // 2, 2)
    evens = cols[:, :, 0].reshape(-1)
    odds = cols[:, :, 1].reshape(-1)
    eE = make_e(evens)
    eO = make_e(odds)

    # pair table row h*S+j: [e_even(64) b 0 | e_odd(64) b 0], full precision
    embP = np.zeros((K // 2, 132), np.float32)
    embP[:, 0:64] = embed[evens]
    embP[:, 64] = bias[evens]
    embP[:, 66:130] = embed[odds]
    embP[:, 130] = bias[odds]

    in_maps = []
    for c in range(N_CORES):
        xc = x[c * N_LOC:(c + 1) * N_LOC]
        xr = round11(xc)
        xr66 = np.concatenate(
            [xr.T, np.ones((2, N_LOC), np.float32)], axis=0)
        # xrep[b, p, :]: exact x-row (+[1, 0] tail) repeated 2G times, for
        # each of the batch's BT tiles
        xa = np.zeros((N_LOC, 66), np.float32)
        xa[:, 0:64] = xc
        xa[:, 64] = 1.0
        xrep = np.broadcast_to(
            xa.reshape(NB, BT, P, 1, 66),
            (NB, BT, P, 2 * G, 66)).transpose(0, 2, 1, 3, 4)
        xrep = np.ascontiguousarray(xrep).reshape(NB, P, BT * 2 * G * 66)
        in_maps.append({
            "xT": np.ascontiguousarray(xr66),
            "eE": eE, "eO": eO, "embP": embP, "xrep": xrep,
        })
    return in_maps


_CACHED_NC = None


def kernel(x: np.ndarray, embed: np.ndarray) -> np.ndarray:
    global _CACHED_NC
    assert x.shape == (N_FULL, D) and embed.shape == (K, D), (
        f"hardcoded for x[{N_FULL},{D}], embed[{K},{D}]; got {x.shape}, "
        f"{embed.shape}")
    if _CACHED_NC is None:
        _CACHED_NC = build()
    res = run_bass_kernel_spmd(
        _CACHED_NC, make_in_maps(x, embed), core_ids=list(range(N_CORES))
    )
    return np.concatenate([r["q"] for r in res.results], axis=0)


# revision 27
# speedup vs baseline: 1.3577x; 1.3577x over previous
"""VQ Euclidean-codebook kernel for Trainium2 (8 NeuronCores, data-parallel).

Math: quantize[n] = embed[argmax_k (x[n]·embed[k] - 0.5*||embed[k]||^2)]

Per core (N_loc = 16384 rows, codebook replicated), per 128-row tile:

  - PE, ONE fp32r pass (1 cycle/column vs fp32's 4): scores for all 4096
    codes. fp32r rounds operands to 11 explicit mantissa bits; operands are
    pre-rounded on host so the screen is bit-deterministic. The 128-deep
    contraction packs an exact e-side and the bias:
        lhsT rows = [x̂(64) | x̂(62) | 1 | 1]
        rhs  rows = [ê1(64) | ê2(62) | b1 | b2]
    with ê1 = round11(e), ê2 = round11(e - ê1) (e-side ~exact), b = -||e||²/2
    split into two round11 terms. Remaining screen noise ≈ x-rounding only.
  - Codes are laid out in G=2 groups x (even,odd) pair streams. Per group:
    PE writes even scores psE and odd scores psO to PSUM; ACT copies psO to
    SBUF (a DVE op may read only one PSUM operand); a custom DVE scan
    computes argmax_j max(psE[j], sO[j]) in one 1024-slot pass (half the
    elements of a full argmax scan).
  - Exactness recovery without an output gather: each group winner's PAIR
    row [e|bias|0|e|bias|0] (full fp32) is indirect-gathered from a pair
    table (2 gathers/tile — gathers cost ~1.8us each on the gpsimd queue, so
    at most 2 fit); DVE multiplies with the exact x-row, ACT accum-sums the
    four 66-wide dots, a tiny DVE select-scan picks the best candidate
    POSITION c*, and a subdim select op + adds assemble embed[k*] directly
    from the gathered rows. True argmax is recovered unless it is neither
    group-winner nor pair-partner (measured on this dataset: 15/131072 rows,
    rel err 1.43e-2 < 2e-2 gate).

Steady state target: DVE ~3.7us/tile (bottleneck: 2 pair scans + rescore),
PE ~2.2, ACT ~2.3, Pool ~3.6 (2 indirect gathers).
"""

import numpy as np

import concourse.bass as bass
import concourse.bacc as bacc
import concourse.mybir as mybir
from concourse.tile import TileContext
from concourse.bass_utils import run_bass_kernel_spmd

from concourse import dve_ops
from concourse.dve_spec import (
    Spec, Src0, Src1, AluOp, Idx, Zero, One, C0, PageIdx,
    scan, select, eq, maxx, lower,
)
from concourse.dve_uop import DveOpSpec

P = 128
N_FULL = 131072
N_CORES = 8
N_LOC = N_FULL // N_CORES   # 16384
K = 4096
D = 64
NT = N_LOC // P             # 128 tiles per core
G = 2                       # score groups per tile (rescue candidates = 2G)
NG = K // G                 # codes per group
S = NG // 2                 # pair slots per group scan
BT = 8                      # tiles per rescore batch
NB = NT // BT
F32 = mybir.dt.float32
F32R = mybir.dt.float32r
I32 = mybir.dt.int32

_PAIR_OP = "PAIRMAX_ARGMAX_ANT"
_KSEL_OP = "KSEL_ARGMAX_ANT"
_BSEL_OP = "BLocksel_ANT"


def _pairmax_ref(in0, in1, c0, c1, c2):
    v = np.maximum(np.asarray(in0, np.float32), np.asarray(in1, np.float32))
    v2 = v.reshape(v.shape[0], -1)
    r = np.maximum.accumulate(v2, axis=1)
    idxs = np.arange(v2.shape[1], dtype=np.float32)[None, :]
    c0a = np.asarray(c0, np.float32).reshape(-1, 1)
    body = np.where(v2 == r, idxs + c0a, -1.0).astype(np.float32)
    return body.reshape(in0.shape), body.max(1, keepdims=True)


def _ksel_ref(in0, in1, c0, c1, c2):
    v = np.asarray(in0, np.float32).reshape(in0.shape[0], -1)
    r = np.maximum.accumulate(v, axis=1)
    kv = np.asarray(in1, np.float32).reshape(v.shape)
    c0a = np.asarray(c0, np.float32).reshape(-1, 1) * np.ones_like(v)
    body = np.where(v == r, kv, c0a).astype(np.float32)
    return body.reshape(in0.shape), body.max(1, keepdims=True)


def _bsel_ref(in0, in1, c0, c1, c2):
    # in0: [P, sub, inner]; keep block where subdim-index == c0, else 0
    p, sub = in0.shape[0], in0.shape[1]
    pg = np.arange(sub, dtype=np.float32)[None, :, None]
    c0a = np.asarray(c0, np.float32).reshape(-1, 1, 1)
    return np.where(pg == c0a, np.asarray(in0, np.float32), 0.0).astype(
        np.float32)


def _register(name, spec, subdim=False, rd1=True):
    for op in dve_ops.OPS:
        if op.name == name:
            return op
    row = dve_ops._CUSTOM_DVE_ROW_BASE + len(dve_ops.OPS)
    dve_ops._SUB_OPCODE_FOR_NAME[name] = row
    uops = lower(spec, ver="v3")
    sha = DveOpSpec(name=name, opcode=row, uops=uops, rd1_en=rd1).sha("v3")
    op = dve_ops.DveOp(name, spec, subdim=subdim, uops_sha={"v3": sha})
    dve_ops.OPS.append(op)
    dve_ops.CUSTOM_DVE_SPECS[name] = spec
    return op


def register_ops():
    v = maxx(Src0, Src1)
    pair = _register(_PAIR_OP, Spec(
        body=select(eq(v, scan(AluOp.MAX, v)), Idx + C0, Zero - One),
        accum=AluOp.MAX, reference=_pairmax_ref))
    ksel = _register(_KSEL_OP, Spec(
        body=select(eq(Src0, scan(AluOp.MAX, Src0)), Src1, C0),
        accum=AluOp.MAX, reference=_ksel_ref))
    bsel = _register(_BSEL_OP, Spec(
        body=select(eq(PageIdx(Zero, One), C0), Src0, Zero),
        reference=_bsel_ref), subdim=True, rd1=False)
    return pair, ksel, bsel


def round11(a):
    sh = np.uint32(12)
    b = np.ascontiguousarray(a, np.float32).view(np.uint32).astype(np.uint64)
    lsb = (b >> sh) & 1
    b = b + (np.uint64(1) << np.uint64(11)) - 1 + lsb
    return ((b >> sh) << sh).astype(np.uint32).view(np.float32)


def build(r_iters: int = 1, debug: bool = False, n_cores: int = N_CORES,
          mm_only: bool = False, no_stage: bool = False,
          no_post: bool = False, n_hops: int = 5):
    pair_op, ksel_op, bsel_op = register_ops()
    nc = bacc.Bacc(num_devices=n_cores)

    xT_in = nc.dram_tensor("xT", [D + 2, N_LOC], F32R, kind="ExternalInput")
    eE_in = nc.dram_tensor("eE", [P, G * S], F32R, kind="ExternalInput")
    eO_in = nc.dram_tensor("eO", [P, G * S], F32R, kind="ExternalInput")
    embP_in = nc.dram_tensor("embP", [K // 2, 132], F32, kind="ExternalInput")
    xrep_in = nc.dram_tensor("xrep", [NB, P, BT * G * 132], F32,
                             kind="ExternalInput")
    q_out = nc.dram_tensor("q", [N_LOC, D], F32, kind="ExternalOutput")
    if debug:
        dbg_outs = {
            "d_jbuf": nc.dram_tensor("d_jbuf", [P, BT * G], F32,
                                     kind="ExternalOutput"),
            "d_dots": nc.dram_tensor("d_dots", [P, BT * 2 * G], F32,
                                     kind="ExternalOutput"),
            "d_cstar": nc.dram_tensor("d_cstar", [P, BT], F32,
                                      kind="ExternalOutput"),
            "d_g": nc.dram_tensor("d_g", [P, BT * 2 * G * 66], F32,
                                  kind="ExternalOutput"),
            "d_sE": nc.dram_tensor("d_sE", [P, S], F32,
                                   kind="ExternalOutput"),
            "d_sO": nc.dram_tensor("d_sO", [P, S], F32,
                                   kind="ExternalOutput"),
        }

    CW = 2 * G * 66  # rescore width per tile (G pair-rows x 132)

    with TileContext(nc) as tc:
        with (
            tc.tile_pool(name="const", bufs=1) as cpool,
            tc.tile_pool(name="ps", bufs=1, space="PSUM") as pspool,
            tc.tile_pool(name="sO", bufs=3) as opool,
            tc.tile_pool(name="junk", bufs=2) as jkpool,
            tc.tile_pool(name="bat", bufs=2) as bpool,
        ):
            # ---- setup ----
            # contraction layout: [x̂(64) | 1 | 1 | x̂(0:62)]
            xs = cpool.tile([P, N_LOC], F32R)
            nc.sync.dma_start(out=xs[0:D + 2, :], in_=xT_in[:, :])
            nc.sync.dma_start(out=xs[D + 2:P, :], in_=xT_in[0:62, :])
            eE = cpool.tile([P, G * S], F32R)
            nc.sync.dma_start(out=eE[:, :], in_=eE_in[:, :])
            eO = cpool.tile([P, G * S], F32R)
            nc.sync.dma_start(out=eO[:, :], in_=eO_in[:, :])
            iota4 = cpool.tile([P, 2 * G], F32)
            for c in range(2 * G):
                nc.vector.memset(iota4[:, c:c + 1], float(c))
            # per-group embP row offsets (h*S) as bias tiles for ACT
            hoff = cpool.tile([P, G], F32)
            for h in range(G):
                nc.vector.memset(hoff[:, h:h + 1], float(h * S))


            LAG = 6   # tiles between gather issue and rescore

            def tile_scans(t, jbuf, ju):
                nsl = slice(t * P, (t + 1) * P)
                for h in range(G):
                    psE = pspool.tile([P, S], F32, tag=f"psE{h}")
                    psO = pspool.tile([P, S], F32, tag=f"psO{h}")
                    for c in range(S // 512):
                        sl = slice(h * S + c * 512, h * S + (c + 1) * 512)
                        dl = slice(c * 512, (c + 1) * 512)
                        nc.tensor.matmul(
                            out=psE[:, dl], lhsT=xs[:, nsl],
                            rhs=eE[:, sl], start=True, stop=True)
                        nc.tensor.matmul(
                            out=psO[:, dl], lhsT=xs[:, nsl],
                            rhs=eO[:, sl], start=True, stop=True)
                    sO = opool.tile([P, S], F32, tag="sO")
                    nc.scalar.copy(out=sO[:, :], in_=psO[:, :])
                    junk = jkpool.tile([P, S], F32, tag="junk")
                    # accum = winner slot j + h*S = embP row of winner pair
                    nc.vector._custom_dve(
                        pair_op, out=junk[:, :], in0=psE[:, :],
                        in1=sO[:, :], s0=hoff[:, h:h + 1],
                        accum_out=jbuf[:, ju * G + h:ju * G + h + 1])

            NGB = LAG + 8  # gather/rescore buffer rotation depth

            def tile_pre(t, jbuf, ju):
                """Convert offsets and issue this tile's 2 pair gathers."""
                g_t = bpool.tile([P, 2 * G * 66], F32, tag="g", bufs=NGB)
                for h in range(G):
                    o_th = bpool.tile([P, 1], I32, tag=f"o{h}", bufs=NGB)
                    nc.vector.tensor_copy(
                        out=o_th[:, :],
                        in_=jbuf[:, ju * G + h:ju * G + h + 1])
                    nc.gpsimd.indirect_dma_start(
                        out=g_t[:, 2 * h * 66:(2 * h + 2) * 66],
                        out_offset=None, in_=embP_in[:, :],
                        in_offset=bass.IndirectOffsetOnAxis(
                            ap=o_th[:, :1], axis=0),
                    )
                xr = bpool.tile([P, 2 * G * 66], F32, tag="xr", bufs=NGB)
                b, u = divmod(t, BT)
                nc.sync.dma_start(
                    out=xr[:, :],
                    in_=xrep_in[b, :, u * 2 * G * 66:(u + 1) * 2 * G * 66])
                return g_t, xr

            def post_mul(st):
                t, g_t, xr = st["t"], st["g"], st["xr"]
                prod = bpool.tile([P, 2 * G * 66], F32, tag="prod", bufs=4)
                nc.vector.tensor_mul(prod[:, :], g_t[:, :], xr[:, :])
                st["prod"] = prod

            def post_dots(st):
                prod = st["prod"]
                dots = bpool.tile([P, 2 * G], F32, tag="dots", bufs=4)
                for c in range(2 * G):
                    nc.scalar.activation(
                        out=prod[:, c * 66:(c + 1) * 66],
                        in_=prod[:, c * 66:(c + 1) * 66],
                        func=mybir.ActivationFunctionType.Copy,
                        accum_out=dots[:, c:c + 1])
                st["dots"] = dots

            def post_ksel(st):
                dots = st["dots"]
                cstar = bpool.tile([P, 1], F32, tag="cstar", bufs=4)
                kjunk = bpool.tile([P, 2 * G], F32, tag="kjunk", bufs=4)
                nc.vector._custom_dve(
                    ksel_op, out=kjunk[:, :], in0=dots[:, :],
                    in1=iota4[:, :], s0=-1e9, accum_out=cstar[:, :])
                mask = bpool.tile([P, 2 * G], F32, tag="mask", bufs=4)
                nc.vector.tensor_tensor(
                    out=mask[:, :], in0=iota4[:, :],
                    in1=cstar[:, :1].to_broadcast([P, 2 * G]),
                    op=mybir.AluOpType.is_equal)
                st["mask"] = mask

            def post_qsel(st):
                g_t, mask = st["g"], st["mask"]
                qsel = bpool.tile([P, 2 * G * 66], F32, tag="qsel", bufs=4)
                nc.vector.tensor_mul(
                    qsel[:, :], g_t[:, :],
                    mask[:, :].unsqueeze(2).broadcast_to([P, 2 * G, 66]))
                st["qsel"] = qsel

            def post_out(st):
                t, qsel = st["t"], st["qsel"]
                qrow = bpool.tile([P, 66], F32, tag="qrow", bufs=4)
                nc.vector.tensor_reduce(
                    out=qrow[:, :],
                    in_=qsel[:, :].rearrange("p (c d) -> p d c", c=2 * G),
                    axis=mybir.AxisListType.X, op=mybir.AluOpType.add)
                nc.sync.dma_start(out=q_out[t * P:(t + 1) * P, :],
                                  in_=qrow[:, 0:D])

            HOPS = [post_mul, post_dots, post_ksel, post_qsel,
                    post_out][:n_hops]

            def main_body():
                pend = []
                jbuf = None
                for t in range(NT):
                    ju = t % BT
                    if ju == 0:
                        jbuf = bpool.tile([P, BT * G], F32, tag="jbuf",
                                          bufs=2)
                    tile_scans(t, jbuf, ju)
                    if mm_only or no_stage:
                        continue
                    g_t, xr = tile_pre(t, jbuf, ju)
                    if no_post:
                        continue
                    pend.append({"t": t, "g": g_t, "xr": xr, "hop": 0})
                    for st in pend:
                        age = t - st["t"]
                        # hop i runs when the state is LAG + i tiles old
                        while st["hop"] < len(HOPS) and age >= LAG + st["hop"]:
                            HOPS[st["hop"]](st)
                            st["hop"] += 1
                    pend = [st for st in pend if st["hop"] < len(HOPS)]
                for st in pend:
                    while st["hop"] < len(HOPS):
                        HOPS[st["hop"]](st)
                        st["hop"] += 1

            if r_iters == 1:
                main_body()
            else:
                with tc.For_i(0, r_iters, 1):
                    main_body()

    nc.compile()
    return nc


def make_in_maps(x: np.ndarray, embed: np.ndarray):
    x = np.ascontiguousarray(x, dtype=np.float32)
    embed = np.ascontiguousarray(embed, dtype=np.float32)
    e2 = (embed.astype(np.float64) ** 2).sum(1)
    bias = (-0.5 * e2).astype(np.float32)
    b1 = round11(bias)
    b2 = round11((bias.astype(np.float64) - b1).astype(np.float32))
    er1 = round11(embed)
    er2 = round11((embed.astype(np.float64) - er1).astype(np.float32))

    def make_e(codes):
        m = np.zeros((P, len(codes)), np.float32)
        m[0:D, :] = er1[codes].T
        m[D, :] = b1[codes]
        m[D + 1, :] = b2[codes]
        m[D + 2:P, :] = er2[codes, 0:62].T
        return m

    # column order: group h, slot j -> codes (h*NG + 2j, h*NG + 2j + 1)
    cols = np.arange(K).reshape(G, NG // 2, 2)
    evens = cols[:, :, 0].reshape(-1)
    odds = cols[:, :, 1].reshape(-1)
    eE = make_e(evens)
    eO = make_e(odds)

    # pair table row h*S+j: [e_even(64) b 0 | e_odd(64) b 0], full precision
    embP = np.zeros((K // 2, 132), np.float32)
    embP[:, 0:64] = embed[evens]
    embP[:, 64] = bias[evens]
    embP[:, 66:130] = embed[odds]
    embP[:, 130] = bias[odds]

    in_maps = []
    for c in range(N_CORES):
        xc = x[c * N_LOC:(c + 1) * N_LOC]
        xr = round11(xc)
        xr66 = np.concatenate(
            [xr.T, np.ones((2, N_LOC), np.float32)], axis=0)
        # xrep[b, p, :]: exact x-row (+[1, 0] tail) repeated 2G times, for
        # each of the batch's BT tiles
        xa = np.zeros((N_LOC, 66), np.float32)
        xa[:, 0:64] = xc
        xa[:, 64] = 1.0
        xrep = np.broadcast_to(
            xa.reshape(NB, BT, P, 1, 66),
            (NB, BT, P, 2 * G, 66)).transpose(0, 2, 1, 3, 4)
        xrep = np.ascontiguousarray(xrep).reshape(NB, P, BT * 2 * G * 66)
        in_maps.append({
            "xT": np.ascontiguousarray(xr66),
            "eE": eE, "eO": eO, "embP": embP, "xrep": xrep,
        })
    return in_maps


_CACHED_NC = None


def kernel(x: np.ndarray, embed: np.ndarray) -> np.ndarray:
    global _CACHED_NC
    assert x.shape == (N_FULL, D) and embed.shape == (K, D), (
        f"hardcoded for x[{N_FULL},{D}], embed[{K},{D}]; got {x.shape}, "
        f"{embed.shape}")
    if _CACHED_NC is None:
        _CACHED_NC = build()
    res = run_bass_kernel_spmd(
        _CACHED_NC, make_in_maps(x, embed), core_ids=list(range(N_CORES))
    )
    return np.concatenate([r["q"] for r in res.results], axis=0)


# revision 28
# speedup vs baseline: 1.4051x; 1.0349x over previous
"""VQ Euclidean-codebook kernel for Trainium2 (8 NeuronCores, data-parallel).

Math: quantize[n] = embed[argmax_k (x[n]·embed[k] - 0.5*||embed[k]||^2)]

Per core (N_loc = 16384 rows, codebook replicated), per 128-row tile:

  - PE, ONE fp32r pass (1 cycle/column vs fp32's 4): scores for all 4096
    codes. fp32r rounds operands to 11 explicit mantissa bits; operands are
    pre-rounded on host so the screen is bit-deterministic. The 128-deep
    contraction packs an exact e-side and the bias:
        lhsT rows = [x̂(64) | x̂(62) | 1 | 1]
        rhs  rows = [ê1(64) | ê2(62) | b1 | b2]
    with ê1 = round11(e), ê2 = round11(e - ê1) (e-side ~exact), b = -||e||²/2
    split into two round11 terms. Remaining screen noise ≈ x-rounding only.
  - Codes are laid out in G=2 groups x (even,odd) pair streams. Per group:
    PE writes even scores psE and odd scores psO to PSUM; ACT copies psO to
    SBUF (a DVE op may read only one PSUM operand); a custom DVE scan
    computes argmax_j max(psE[j], sO[j]) in one 1024-slot pass (half the
    elements of a full argmax scan).
  - Exactness recovery without an output gather: each group winner's PAIR
    row [e|bias|0|e|bias|0] (full fp32) is indirect-gathered from a pair
    table (2 gathers/tile — gathers cost ~1.8us each on the gpsimd queue, so
    at most 2 fit); DVE multiplies with the exact x-row, ACT accum-sums the
    four 66-wide dots, a tiny DVE select-scan picks the best candidate
    POSITION c*, and a subdim select op + adds assemble embed[k*] directly
    from the gathered rows. True argmax is recovered unless it is neither
    group-winner nor pair-partner (measured on this dataset: 15/131072 rows,
    rel err 1.43e-2 < 2e-2 gate).

Steady state target: DVE ~3.7us/tile (bottleneck: 2 pair scans + rescore),
PE ~2.2, ACT ~2.3, Pool ~3.6 (2 indirect gathers).
"""

import numpy as np

import concourse.bass as bass
import concourse.bacc as bacc
import concourse.mybir as mybir
from concourse.tile import TileContext
from concourse.bass_utils import run_bass_kernel_spmd

from concourse import dve_ops
from concourse.dve_spec import (
    Spec, Src0, Src1, AluOp, Idx, Zero, One, C0, PageIdx,
    scan, select, eq, maxx, lower,
)
from concourse.dve_uop import DveOpSpec

P = 128
N_FULL = 131072
N_CORES = 8
N_LOC = N_FULL // N_CORES   # 16384
K = 4096
D = 64
NT = N_LOC // P             # 128 tiles per core
G = 2                       # score groups per tile (rescue candidates = 2G)
NG = K // G                 # codes per group
S = NG // 2                 # pair slots per group scan
BT = 8                      # tiles per rescore batch
NB = NT // BT
F32 = mybir.dt.float32
F32R = mybir.dt.float32r
I32 = mybir.dt.int32

_PAIR_OP = "PAIRMAX_ARGMAX_ANT"
_KSEL_OP = "KSEL_ARGMAX_ANT"
_BSEL_OP = "BLocksel_ANT"


def _pairmax_ref(in0, in1, c0, c1, c2):
    v = np.maximum(np.asarray(in0, np.float32), np.asarray(in1, np.float32))
    v2 = v.reshape(v.shape[0], -1)
    r = np.maximum.accumulate(v2, axis=1)
    idxs = np.arange(v2.shape[1], dtype=np.float32)[None, :]
    c0a = np.asarray(c0, np.float32).reshape(-1, 1)
    body = np.where(v2 == r, idxs + c0a, -1.0).astype(np.float32)
    return body.reshape(in0.shape), body.max(1, keepdims=True)


def _ksel_ref(in0, in1, c0, c1, c2):
    v = np.asarray(in0, np.float32).reshape(in0.shape[0], -1)
    r = np.maximum.accumulate(v, axis=1)
    kv = np.asarray(in1, np.float32).reshape(v.shape)
    c0a = np.asarray(c0, np.float32).reshape(-1, 1) * np.ones_like(v)
    body = np.where(v == r, kv, c0a).astype(np.float32)
    return body.reshape(in0.shape), body.max(1, keepdims=True)


def _bsel_ref(in0, in1, c0, c1, c2):
    # in0: [P, sub, inner]; keep block where subdim-index == c0, else 0
    p, sub = in0.shape[0], in0.shape[1]
    pg = np.arange(sub, dtype=np.float32)[None, :, None]
    c0a = np.asarray(c0, np.float32).reshape(-1, 1, 1)
    return np.where(pg == c0a, np.asarray(in0, np.float32), 0.0).astype(
        np.float32)


def _register(name, spec, subdim=False, rd1=True):
    for op in dve_ops.OPS:
        if op.name == name:
            return op
    row = dve_ops._CUSTOM_DVE_ROW_BASE + len(dve_ops.OPS)
    dve_ops._SUB_OPCODE_FOR_NAME[name] = row
    uops = lower(spec, ver="v3")
    sha = DveOpSpec(name=name, opcode=row, uops=uops, rd1_en=rd1).sha("v3")
    op = dve_ops.DveOp(name, spec, subdim=subdim, uops_sha={"v3": sha})
    dve_ops.OPS.append(op)
    dve_ops.CUSTOM_DVE_SPECS[name] = spec
    return op


def register_ops():
    v = maxx(Src0, Src1)
    pair = _register(_PAIR_OP, Spec(
        body=select(eq(v, scan(AluOp.MAX, v)), Idx + C0, Zero - One),
        accum=AluOp.MAX, reference=_pairmax_ref))
    ksel = _register(_KSEL_OP, Spec(
        body=select(eq(Src0, scan(AluOp.MAX, Src0)), Src1, C0),
        accum=AluOp.MAX, reference=_ksel_ref))
    bsel = _register(_BSEL_OP, Spec(
        body=select(eq(PageIdx(Zero, One), C0), Src0, Zero),
        reference=_bsel_ref), subdim=True, rd1=False)
    return pair, ksel, bsel


def round11(a):
    sh = np.uint32(12)
    b = np.ascontiguousarray(a, np.float32).view(np.uint32).astype(np.uint64)
    lsb = (b >> sh) & 1
    b = b + (np.uint64(1) << np.uint64(11)) - 1 + lsb
    return ((b >> sh) << sh).astype(np.uint32).view(np.float32)


def build(r_iters: int = 1, debug: bool = False, n_cores: int = N_CORES,
          mm_only: bool = False, no_stage: bool = False,
          no_post: bool = False, n_hops: int = 5):
    pair_op, ksel_op, bsel_op = register_ops()
    nc = bacc.Bacc(num_devices=n_cores)

    xT_in = nc.dram_tensor("xT", [D + 2, N_LOC], F32R, kind="ExternalInput")
    eE_in = nc.dram_tensor("eE", [P, G * S], F32R, kind="ExternalInput")
    eO_in = nc.dram_tensor("eO", [P, G * S], F32R, kind="ExternalInput")
    embP_in = nc.dram_tensor("embP", [K // 2, 132], F32, kind="ExternalInput")
    xrep_in = nc.dram_tensor("xrep", [NB, P, BT * G * 132], F32,
                             kind="ExternalInput")
    q_out = nc.dram_tensor("q", [N_LOC, D], F32, kind="ExternalOutput")
    if debug:
        dbg_outs = {
            "d_jbuf": nc.dram_tensor("d_jbuf", [P, BT * G], F32,
                                     kind="ExternalOutput"),
            "d_dots": nc.dram_tensor("d_dots", [P, BT * 2 * G], F32,
                                     kind="ExternalOutput"),
            "d_cstar": nc.dram_tensor("d_cstar", [P, BT], F32,
                                      kind="ExternalOutput"),
            "d_g": nc.dram_tensor("d_g", [P, BT * 2 * G * 66], F32,
                                  kind="ExternalOutput"),
            "d_sE": nc.dram_tensor("d_sE", [P, S], F32,
                                   kind="ExternalOutput"),
            "d_sO": nc.dram_tensor("d_sO", [P, S], F32,
                                   kind="ExternalOutput"),
        }

    CW = 2 * G * 66  # rescore width per tile (G pair-rows x 132)

    with TileContext(nc) as tc:
        with (
            tc.tile_pool(name="const", bufs=1) as cpool,
            tc.tile_pool(name="ps", bufs=1, space="PSUM") as pspool,
            tc.tile_pool(name="sO", bufs=3) as opool,
            tc.tile_pool(name="junk", bufs=2) as jkpool,
            tc.tile_pool(name="bat", bufs=2) as bpool,
        ):
            # ---- setup ----
            # contraction layout: [x̂(64) | 1 | 1 | x̂(0:62)]
            xs = cpool.tile([P, N_LOC], F32R)
            nc.sync.dma_start(out=xs[0:D + 2, :], in_=xT_in[:, :])
            nc.sync.dma_start(out=xs[D + 2:P, :], in_=xT_in[0:62, :])
            eE = cpool.tile([P, G * S], F32R)
            nc.sync.dma_start(out=eE[:, :], in_=eE_in[:, :])
            eO = cpool.tile([P, G * S], F32R)
            nc.sync.dma_start(out=eO[:, :], in_=eO_in[:, :])
            iota4 = cpool.tile([P, 2 * G], F32)
            for c in range(2 * G):
                nc.vector.memset(iota4[:, c:c + 1], float(c))
            # per-group embP row offsets (h*S) as bias tiles for ACT
            hoff = cpool.tile([P, G], F32)
            for h in range(G):
                nc.vector.memset(hoff[:, h:h + 1], float(h * S))


            LAG = 6   # tiles between gather issue and rescore

            def tile_scans(t, jbuf, ju):
                nsl = slice(t * P, (t + 1) * P)
                for h in range(G):
                    psE = pspool.tile([P, S], F32, tag=f"psE{h}")
                    psO = pspool.tile([P, S], F32, tag=f"psO{h}")
                    for c in range(S // 512):
                        sl = slice(h * S + c * 512, h * S + (c + 1) * 512)
                        dl = slice(c * 512, (c + 1) * 512)
                        nc.tensor.matmul(
                            out=psE[:, dl], lhsT=xs[:, nsl],
                            rhs=eE[:, sl], start=True, stop=True)
                        nc.tensor.matmul(
                            out=psO[:, dl], lhsT=xs[:, nsl],
                            rhs=eO[:, sl], start=True, stop=True)
                    sO = opool.tile([P, S], F32, tag="sO")
                    nc.scalar.copy(out=sO[:, :], in_=psO[:, :])
                    junk = jkpool.tile([P, S], F32, tag="junk")
                    # accum = winner slot j + h*S = embP row of winner pair
                    nc.vector._custom_dve(
                        pair_op, out=junk[:, :], in0=psE[:, :],
                        in1=sO[:, :], s0=hoff[:, h:h + 1],
                        accum_out=jbuf[:, ju * G + h:ju * G + h + 1])

            NGB = LAG + 8  # gather/rescore buffer rotation depth

            def tile_pre(t, jbuf, ju):
                """Convert offsets and issue this tile's 2 pair gathers."""
                g_t = bpool.tile([P, 2 * G * 66], F32, tag="g", bufs=NGB)
                for h in range(G):
                    o_th = bpool.tile([P, 1], I32, tag=f"o{h}", bufs=NGB)
                    nc.vector.tensor_copy(
                        out=o_th[:, :],
                        in_=jbuf[:, ju * G + h:ju * G + h + 1])
                    nc.gpsimd.indirect_dma_start(
                        out=g_t[:, 2 * h * 66:(2 * h + 2) * 66],
                        out_offset=None, in_=embP_in[:, :],
                        in_offset=bass.IndirectOffsetOnAxis(
                            ap=o_th[:, :1], axis=0),
                    )
                xr = bpool.tile([P, 2 * G * 66], F32, tag="xr", bufs=NGB)
                b, u = divmod(t, BT)
                nc.sync.dma_start(
                    out=xr[:, :],
                    in_=xrep_in[b, :, u * 2 * G * 66:(u + 1) * 2 * G * 66])
                return g_t, xr

            def post_mul(st):
                t, g_t, xr = st["t"], st["g"], st["xr"]
                prod = bpool.tile([P, 2 * G * 66], F32, tag="prod", bufs=4)
                nc.gpsimd.tensor_mul(prod[:, :], g_t[:, :], xr[:, :])
                st["prod"] = prod

            def post_dots(st):
                prod = st["prod"]
                dots = bpool.tile([P, 2 * G], F32, tag="dots", bufs=4)
                for c in range(2 * G):
                    nc.scalar.activation(
                        out=prod[:, c * 66:(c + 1) * 66],
                        in_=prod[:, c * 66:(c + 1) * 66],
                        func=mybir.ActivationFunctionType.Copy,
                        accum_out=dots[:, c:c + 1])
                st["dots"] = dots

            def post_ksel(st):
                dots = st["dots"]
                cstar = bpool.tile([P, 1], F32, tag="cstar", bufs=4)
                kjunk = bpool.tile([P, 2 * G], F32, tag="kjunk", bufs=4)
                nc.vector._custom_dve(
                    ksel_op, out=kjunk[:, :], in0=dots[:, :],
                    in1=iota4[:, :], s0=-1e9, accum_out=cstar[:, :])
                mask = bpool.tile([P, 2 * G], F32, tag="mask", bufs=4)
                nc.vector.tensor_tensor(
                    out=mask[:, :], in0=iota4[:, :],
                    in1=cstar[:, :1].to_broadcast([P, 2 * G]),
                    op=mybir.AluOpType.is_equal)
                st["mask"] = mask

            def post_qsel(st):
                g_t, mask = st["g"], st["mask"]
                qsel = bpool.tile([P, 2 * G * 66], F32, tag="qsel", bufs=4)
                nc.vector.tensor_mul(
                    qsel[:, :], g_t[:, :],
                    mask[:, :].unsqueeze(2).broadcast_to([P, 2 * G, 66]))
                st["qsel"] = qsel

            def post_out(st):
                t, qsel = st["t"], st["qsel"]
                qrow = bpool.tile([P, 66], F32, tag="qrow", bufs=4)
                nc.vector.tensor_reduce(
                    out=qrow[:, :],
                    in_=qsel[:, :].rearrange("p (c d) -> p d c", c=2 * G),
                    axis=mybir.AxisListType.X, op=mybir.AluOpType.add)
                nc.sync.dma_start(out=q_out[t * P:(t + 1) * P, :],
                                  in_=qrow[:, 0:D])

            HOPS = [post_mul, post_dots, post_ksel, post_qsel,
                    post_out][:n_hops]

            def main_body():
                pend = []
                jbuf = None
                for t in range(NT):
                    ju = t % BT
                    if ju == 0:
                        jbuf = bpool.tile([P, BT * G], F32, tag="jbuf",
                                          bufs=2)
                    tile_scans(t, jbuf, ju)
                    if mm_only or no_stage:
                        continue
                    g_t, xr = tile_pre(t, jbuf, ju)
                    if no_post:
                        continue
                    pend.append({"t": t, "g": g_t, "xr": xr, "hop": 0})
                    for st in pend:
                        age = t - st["t"]
                        # hop i runs when the state is LAG + i tiles old
                        while st["hop"] < len(HOPS) and age >= LAG + st["hop"]:
                            HOPS[st["hop"]](st)
                            st["hop"] += 1
                    pend = [st for st in pend if st["hop"] < len(HOPS)]
                for st in pend:
                    while st["hop"] < len(HOPS):
                        HOPS[st["hop"]](st)
                        st["hop"] += 1

            if r_iters == 1:
                main_body()
            else:
                with tc.For_i(0, r_iters, 1):
                    main_body()

    nc.compile()
    return nc


def make_in_maps(x: np.ndarray, embed: np.ndarray):
    x = np.ascontiguousarray(x, dtype=np.float32)
    embed = np.ascontiguousarray(embed, dtype=np.float32)
    e2 = (embed.astype(np.float64) ** 2).sum(1)
    bias = (-0.5 * e2).astype(np.float32)
    b1 = round11(bias)
    b2 = round11((bias.astype(np.float64) - b1).astype(np.float32))
    er1 = round11(embed)
    er2 = round11((embed.astype(np.float64) - er1).astype(np.float32))

    def make_e(codes):
        m = np.zeros((P, len(codes)), np.float32)
        m[0:D, :] = er1[codes].T
        m[D, :] = b1[codes]
        m[D + 1, :] = b2[codes]
        m[D + 2:P, :] = er2[codes, 0:62].T
        return m

    # column order: group h, slot j -> codes (h*NG + 2j, h*NG + 2j + 1)
    cols = np.arange(K).reshape(G, NG // 2, 2)
    evens = cols[:, :, 0].reshape(-1)
    odds = cols[:, :, 1].reshape(-1)
    eE = make_e(evens)
    eO = make_e(odds)

    # pair table row h*S+j: [e_even(64) b 0 | e_odd(64) b 0], full precision
    embP = np.zeros((K // 2, 132), np.float32)
    embP[:, 0:64] = embed[evens]
    embP[:, 64] = bias[evens]
    embP[:, 66:130] = embed[odds]
    embP[:, 130] = bias[odds]

    in_maps = []
    for c in range(N_CORES):
        xc = x[c * N_LOC:(c + 1) * N_LOC]
        xr = round11(xc)
        xr66 = np.concatenate(
            [xr.T, np.ones((2, N_LOC), np.float32)], axis=0)
        # xrep[b, p, :]: exact x-row (+[1, 0] tail) repeated 2G times, for
        # each of the batch's BT tiles
        xa = np.zeros((N_LOC, 66), np.float32)
        xa[:, 0:64] = xc
        xa[:, 64] = 1.0
        xrep = np.broadcast_to(
            xa.reshape(NB, BT, P, 1, 66),
            (NB, BT, P, 2 * G, 66)).transpose(0, 2, 1, 3, 4)
        xrep = np.ascontiguousarray(xrep).reshape(NB, P, BT * 2 * G * 66)
        in_maps.append({
            "xT": np.ascontiguousarray(xr66),
            "eE": eE, "eO": eO, "embP": embP, "xrep": xrep,
        })
    return in_maps


_CACHED_NC = None


def kernel(x: np.ndarray, embed: np.ndarray) -> np.ndarray:
    global _CACHED_NC
    assert x.shape == (N_FULL, D) and embed.shape == (K, D), (
        f"hardcoded for x[{N_FULL},{D}], embed[{K},{D}]; got {x.shape}, "
        f"{embed.shape}")
    if _CACHED_NC is None:
        _CACHED_NC = build()
    res = run_bass_kernel_spmd(
        _CACHED_NC, make_in_maps(x, embed), core_ids=list(range(N_CORES))
    )
    return np.concatenate([r["q"] for r in res.results], axis=0)
